# revision 21
# baseline (speedup 1.0000x reference)
"""Trainium2 Bass kernel: depthwise 3x3 stencil conv (SAME, zero-pad) + residual.

Math (per image, per channel):
    out[h,w] = sum_{dh,dw} k[dh,dw] * x[h+dh-1, w+dw-1]  +  x[h,w]

The fixed stencil k = [[1,0,-1],[0,1,0],[-1,0,1]] is rank-2:
    k = outer((1,0,-1),(1,0,-1)) + center(1)
so with t[h,w] = x[h-1,w] - x[h+1,w] (vertical pass):
    out[h,w] = 2*x[h,w] + t[h,w-1] - t[h,w+1]

Mapping on one NeuronCore (batch is sharded 4 images/core across 8 cores):
  - layout: partitions = h (112 rows), free dim = (w,c) flattened (10752 f32)
    with 96-float zero pads on both ends (one w column, padded host-side)
  - vertical pass: banded 112x112 matmul on TensorE (PSUM, N=512 chunks)
  - PSUM -> SBUF t-slab copies on ScalarE
  - horizontal pass: two fused in-place DVE ops per piece:
        v   = 2*x + t@(w-1)      (scalar_tensor_tensor)
        out = v - t@(w+1)        (tensor_tensor)
  - straight contiguous HBM DMAs in/out (HWDGE)

fp32 self-loading matmuls can carry only ~1 semaphore wait (single EVENTS
slot in the LDWEIGHTS ISA struct), so DMA-completion waits are absorbed by
tiny dummy matmuls that read one column of the freshly loaded tile.
"""

import sys
import numpy as np

for _p in ("/opt/trn_rl_repo",):
    if _p not in sys.path:
        sys.path.insert(0, _p)

# ---------------- problem constants (hardcoded per contract) ----------------
N_CORES = 8
N, H, W, CH = 32, 112, 112, 96
IMGS_PER_CORE = N // N_CORES          # 4
ROWS = IMGS_PER_CORE * H              # 448 rows per core shard
FS = W * CH                           # 10752 floats per row
PAD = CH                              # one w column of zero padding
SLAB = FS + 2 * PAD                   # 10944
MM_N = 512                            # one PSUM bank of fp32
N_PIECES = 3                          # DVE piece split of the interior
PIECE = FS // N_PIECES                # 3584

_CACHE = {}
LAST_RESULTS = None  # BassKernelResults of the most recent run (for test.py)


def _build_bass(beta):
    """Raw-bass program with a hand-rolled static schedule.

    The walrus codegen used on this toolchain supports at most ONE semaphore
    wait per instruction, which rules out Tile's auto-generated multi-wait
    instructions.  Raw bass emits each wait as its own standalone wait_ge
    instruction on the consuming engine, which is always legal.

    Work is split into 8 units (4 images x 2 w-halves) with 4-deep slab
    buffering so load / matmul / copy / vector / store stages of different
    units overlap.  Per unit u:
        SP :  D(u)  x rows, w-halo cols -> xs[u%4]   (HBM -> SBUF, 2.4 MB)
        PE :  per group g, two accumulating float32r matmuls produce the
              whole shifted-stencil term in PSUM:
                ps[bank] = V^T @ xs[:, g+0]  +  (-V)^T @ xs[:, g+192]
                         = t@(w-1) - t@(w+1)
        ACT:  cp(u,g) ts[u%4][:, g] <- ps[bank]      (PSUM -> SBUF)
        DVE:  op   ts[0:5376] = beta * xs[96:5472] + ts[0:5376]   (exact fp32)
              drain -> inc dve sem
        SP :  O(u)  ts[0:5376] -> out rows/cols      (SBUF -> HBM)

    The matmuls run in float32r (single-pass fp32, ~1e-4 relative error on
    the stencil term, 4x faster than strict fp32 on the PE); the dominant
    residual/center term beta*x stays exact fp32 on the DVE.
    """
    from concourse import bass, mybir

    f32 = mybir.dt.float32
    f32r = mybir.dt.float32r
    nc = bass.Bass(debug=False)
    x_d = nc.declare_dram_parameter("x", [ROWS, SLAB], f32r, isOutput=False)
    v_d = nc.declare_dram_parameter("vmat", [H, H], f32r, isOutput=False)
    vn_d = nc.declare_dram_parameter("vmatn", [H, H], f32r, isOutput=False)
    out_d = nc.declare_dram_parameter("out", [ROWS, FS], f32, isOutput=True)

    WHALF = W // 2            # 56 output columns per unit
    USLAB = (WHALF + 2) * CH  # 5568 slab floats (1 w-col halo each side)
    UINT = WHALF * CH         # 5376 interior floats
    NU = IMGS_PER_CORE * 2    # 8 units
    NS = 4                    # slab sets in flight

    groups = []
    off = 0
    while off < UINT:
        n = min(MM_N, UINT - off)
        groups.append((off, n))
        off += n
    n_g = len(groups)  # 11 (10x512 + 256)

    vt = nc.alloc_sbuf_tensor("vt", [H, H], f32r)
    vtn = nc.alloc_sbuf_tensor("vtn", [H, H], f32r)
    xs = [nc.alloc_sbuf_tensor(f"xs{k}", [H, USLAB], f32r) for k in range(NS)]
    ts = [nc.alloc_sbuf_tensor(f"ts{k}", [H, USLAB], f32) for k in range(NS)]
    NB = 8
    ps = [nc.alloc_psum_tensor(f"ps{b}", [H, MM_N], f32) for b in range(NB)]

    def unit_rows(u):
        i = u // 2
        return i * H, (i + 1) * H

    def unit_slab_col(u):
        # start column of the unit's slab inside the padded x row [ROWS, SLAB]
        return (u % 2) * WHALF * CH  # 0 or 5376

    from contextlib import ExitStack

    with (
        nc.Block(no_gpsimd_drain=True) as block,
        nc.semaphore("s_vt") as s_vt,
        nc.semaphore("s_pe") as s_pe,
        nc.semaphore("s_act") as s_act,
        nc.semaphore("s_dve") as s_dve,
        ExitStack() as _sems,
    ):
        # Per-slab-set DMA completion semaphores.  A single cumulative DMA
        # semaphore would race: concurrent DMAs can complete out of issue
        # order, so "sem >= 16*(u+1)" could be satisfied by a LATER unit's
        # transfer while unit u's data is still in flight.  Per-set sems are
        # safe because successive users of one set never overlap in flight.
        s_din = [_sems.enter_context(nc.semaphore(f"s_din{k}")) for k in range(NS)]
        s_dout = [_sems.enter_context(nc.semaphore(f"s_dout{k}")) for k in range(NS)]

        @block.sync
        def _(sp: bass.BassEngine):
            sp.dma_start(out=vt[:, :], in_=v_d[:, :]).then_inc(s_vt, 16)
            sp.dma_start(out=vtn[:, :], in_=vn_d[:, :]).then_inc(s_vt, 16)

            def load(u):
                r0, r1 = unit_rows(u)
                c0 = unit_slab_col(u)
                sp.dma_start(
                    out=xs[u % NS][:, :], in_=x_d[r0:r1, c0 : c0 + USLAB]
                ).then_inc(s_din[u % NS], 16)

            for u in range(min(NS, NU)):
                load(u)
            UH = UINT // 2  # store/DVE half-unit granularity
            for u in range(NU):
                r0, r1 = unit_rows(u)
                oc0 = (u % 2) * UINT
                for h in range(2):
                    # store half h of unit u once its DVE drain fired
                    sp.wait_ge(s_dve, 2 * u + h + 1)
                    sp.dma_start(
                        out=out_d[r0:r1, oc0 + h * UH : oc0 + (h + 1) * UH],
                        in_=ts[u % NS][:, h * UH : (h + 1) * UH],
                    ).then_inc(s_dout[u % NS], 16)
                nxt = u + NS
                if nxt < NU:
                    # reload xs[u%NS]: PE reads of unit u must be done (DVE
                    # covered by the store wait above)
                    sp.wait_ge(s_pe, n_g * (u + 1))
                    load(nxt)
            for k in range(NS):
                sp.wait_ge(s_dout[k], 32 * (NU // NS))

        @block.tensor
        def _(pe: bass.BassEngine):
            pe.wait_ge(s_vt, 32)
            # warm the PE clock gate (HAM) while the first loads are in
            # flight; results land in bank NB-1 and are overwritten later
            for _w in range(15):
                pe.matmul(
                    out=ps[NB - 1][0:H, 0:H],
                    lhsT=vt[:, :],
                    rhs=vt[:, :],
                    start=True,
                    stop=True,
                )
            for u in range(NU):
                pe.wait_ge(s_din[u % NS], 16 * (u // NS + 1))
                for g, (goff, gn) in enumerate(groups):
                    idx = u * n_g + g
                    if idx >= NB:
                        # psum bank reuse: the copy that read it must be done
                        pe.wait_ge(s_act, idx - NB + 1)
                    # ps = V^T @ x(w-1)  -  V^T @ x(w+1)  =  t@(w-1) - t@(w+1)
                    pe.matmul(
                        out=ps[idx % NB][0:H, 0:gn],
                        lhsT=vt[:, :],
                        rhs=xs[u % NS][:, goff : goff + gn],
                        start=True,
                        stop=False,
                    )
                    pe.matmul(
                        out=ps[idx % NB][0:H, 0:gn],
                        lhsT=vtn[:, :],
                        rhs=xs[u % NS][:, goff + 2 * PAD : goff + 2 * PAD + gn],
                        start=False,
                        stop=True,
                    ).then_inc(s_pe, 1)

        @block.scalar
        def _(act: bass.BassEngine):
            for u in range(NU):
                if u >= NS:
                    # ts slab reuse: unit u-NS's DVE write and store DMA done
                    act.wait_ge(s_dve, 2 * (u - NS) + 2)
                    act.wait_ge(s_dout[u % NS], 32 * (u // NS))
                for g, (goff, gn) in enumerate(groups):
                    idx = u * n_g + g
                    act.wait_ge(s_pe, idx + 1)
                    act.copy(
                        out=ts[u % NS][:, goff : goff + gn],
                        in_=ps[idx % NB][0:H, 0:gn],
                    ).then_inc(s_act, 1)

        @block.vector
        def _(dve: bass.BassEngine):
            UH = UINT // 2
            for u in range(NU):
                for h in range(2):
                    # copies covering this half must have produced ts
                    # (transitively: matmuls and the load are done too; the
                    # op only READS xs)
                    need = (h + 1) * UH
                    dve.wait_ge(s_act, u * n_g + (need + MM_N - 1) // MM_N)
                    dve.scalar_tensor_tensor(
                        out=ts[u % NS][:, h * UH : (h + 1) * UH],
                        in0=xs[u % NS][:, PAD + h * UH : PAD + (h + 1) * UH].bitcast(
                            f32
                        ),
                        scalar=float(beta),
                        in1=ts[u % NS][:, h * UH : (h + 1) * UH],
                        op0=mybir.AluOpType.mult,
                        op1=mybir.AluOpType.add,
                    )
                    dve.drain().then_inc(s_dve, 1)

    return nc


def _stencil_params(kern):
    """Validate the depthwise kernel and extract (vertical profile a, beta).

    Requires: channels identical, k[:,2] == -k[:,0], k[0,1] == k[2,1] == 0.
    Returns (a, beta) with a = k[:,0] (vertical mixing profile) and
    beta = k[1,1] + 1 (center coefficient incl. the residual).
    """
    k = np.asarray(kern, dtype=np.float32)
    if k.ndim != 4 or k.shape != (3, 3, 1, CH):
        return None
    if not np.all(k == k[:, :, :, :1]):
        return None
    k2 = k[:, :, 0, 0]
    if not (np.all(k2[:, 2] == -k2[:, 0]) and k2[0, 1] == 0 and k2[2, 1] == 0):
        return None
    return k2[:, 0].copy(), float(k2[1, 1]) + 1.0


def _numpy_fallback(x, kern):
    """Straightforward shifted-add implementation (safety net only)."""
    k = np.asarray(kern, dtype=np.float32)[:, :, 0, :]  # (3,3,CH)
    xp = np.pad(x, ((0, 0), (1, 1), (1, 1), (0, 0)))
    out = x.astype(np.float32).copy()
    for dh in range(3):
        for dw in range(3):
            out += k[dh, dw] * xp[:, dh : dh + H, dw : dw + W, :]
    return out


def _ensure_ntff_hook():
    """The agent image's antenv lacks axon_hooks; synthesize it so
    run_bass_kernel_spmd(trace=True) can reach the NTFF profiler."""
    import types

    if "antenv.axon_hooks" in sys.modules:
        return
    import antenv

    mod = types.ModuleType("antenv.axon_hooks")
    state = {}
    mod.set_axon_ntff_profile_hook = lambda h: state.__setitem__("h", h)
    mod.get_axon_ntff_profile_hook = lambda: state.get("h")
    sys.modules["antenv.axon_hooks"] = mod
    antenv.axon_hooks = mod
    try:
        if "/root/.axon_site" not in sys.path:
            sys.path.insert(0, "/root/.axon_site")
        from trn_agent_boot.trn_boot import _ntff_profile_via_ctypes

        hook = _ntff_profile_via_ctypes("/opt/axon/libaxon_pjrt.so")
        if hook is not None:
            mod.set_axon_ntff_profile_hook(hook)
    except Exception:
        pass


def _run_on_hw(x, a, beta, trace=False):
    global LAST_RESULTS
    if trace:
        _ensure_ntff_hook()
    from concourse.bass_utils import run_bass_kernel_spmd

    # vertical banded matrix: V[i, j] = coeff of x-row i in t-row j
    V = np.zeros((H, H), dtype=np.float32)
    idx = np.arange(H)
    V[idx[:-1] + 1, idx[:-1]] += a[2]   # i = j+1
    V[idx, idx] += a[1]                 # i = j
    V[idx[1:] - 1, idx[1:]] += a[0]     # i = j-1

    key = (a.tobytes(), float(beta))
    if key not in _CACHE:
        _CACHE[key] = _build_bass(beta)
    nc = _CACHE[key]

    # host-side zero padding of one w column on each side (pads the slab so
    # the device needs no memsets)
    xp = np.zeros((N_CORES, ROWS, SLAB), dtype=np.float32)
    xp[:, :, PAD : PAD + FS] = x.reshape(N_CORES, ROWS, FS)
    Vn = np.ascontiguousarray(-V)
    in_maps = [{"x": xp[c], "vmat": V, "vmatn": Vn} for c in range(N_CORES)]
    res = run_bass_kernel_spmd(nc, in_maps, list(range(N_CORES)), trace=trace)
    LAST_RESULTS = res
    out = np.stack([res.results[c]["out"] for c in range(N_CORES)])
    return out.reshape(N, H, W, CH)


def kernel(x, kernel=None, _trace=False, **_unused):
    x = np.ascontiguousarray(np.asarray(x, dtype=np.float32))
    assert x.shape == (N, H, W, CH), f"unexpected x shape {x.shape}"
    if kernel is None:
        base = np.array(
            [[1.0, 0.0, -1.0], [0.0, 1.0, 0.0], [-1.0, 0.0, 1.0]], dtype=np.float32
        )
        kernel = np.tile(base[:, :, None, None], (1, 1, 1, CH))
    params = _stencil_params(kernel)
    if params is None:
        return _numpy_fallback(x, kernel)
    a, beta = params
    return _run_on_hw(x, a, beta, trace=_trace)


if __name__ == "__main__":
    xs = np.random.randn(N, H, W, CH).astype(np.float32)
    out = kernel(xs)
    print(out.shape, out.dtype)


# revision 25
# speedup vs baseline: 1.0741x; 1.0741x over previous
"""Trainium2 Bass kernel: depthwise 3x3 stencil conv (SAME, zero-pad) + residual.

Math (per image, per channel):
    out[h,w] = sum_{dh,dw} k[dh,dw] * x[h+dh-1, w+dw-1]  +  x[h,w]

The fixed stencil k = [[1,0,-1],[0,1,0],[-1,0,1]] is rank-2:
    k = outer((1,0,-1),(1,0,-1)) + center(1)
so with t[h,w] = x[h-1,w] - x[h+1,w] (vertical pass):
    out[h,w] = 2*x[h,w] + t[h,w-1] - t[h,w+1]

Mapping on one NeuronCore (batch is sharded 4 images/core across 8 cores):
  - layout: partitions = h (112 rows), free dim = (w,c) flattened (10752 f32)
    with 96-float zero pads on both ends (one w column, padded host-side)
  - vertical pass: banded 112x112 matmul on TensorE (PSUM, N=512 chunks)
  - PSUM -> SBUF t-slab copies on ScalarE
  - horizontal pass: two fused in-place DVE ops per piece:
        v   = 2*x + t@(w-1)      (scalar_tensor_tensor)
        out = v - t@(w+1)        (tensor_tensor)
  - straight contiguous HBM DMAs in/out (HWDGE)

fp32 self-loading matmuls can carry only ~1 semaphore wait (single EVENTS
slot in the LDWEIGHTS ISA struct), so DMA-completion waits are absorbed by
tiny dummy matmuls that read one column of the freshly loaded tile.
"""

import sys
import numpy as np

for _p in ("/opt/trn_rl_repo",):
    if _p not in sys.path:
        sys.path.insert(0, _p)

# ---------------- problem constants (hardcoded per contract) ----------------
N_CORES = 8
N, H, W, CH = 32, 112, 112, 96
IMGS_PER_CORE = N // N_CORES          # 4
ROWS = IMGS_PER_CORE * H              # 448 rows per core shard
FS = W * CH                           # 10752 floats per row
PAD = CH                              # one w column of zero padding
SLAB = FS + 2 * PAD                   # 10944
MM_N = 512                            # one PSUM bank of fp32
N_PIECES = 3                          # DVE piece split of the interior
PIECE = FS // N_PIECES                # 3584

_CACHE = {}
LAST_RESULTS = None  # BassKernelResults of the most recent run (for test.py)


def _build_bass(beta):
    """Raw-bass program with a hand-rolled static schedule.

    The walrus codegen used on this toolchain supports at most ONE semaphore
    wait per instruction, which rules out Tile's auto-generated multi-wait
    instructions.  Raw bass emits each wait as its own standalone wait_ge
    instruction on the consuming engine, which is always legal.

    Work is split into 8 units (4 images x 2 w-halves) with 4-deep slab
    buffering so load / matmul / copy / vector / store stages of different
    units overlap.  Per unit u:
        SP :  D(u)  x rows, w-halo cols -> xs[u%4]   (HBM -> SBUF, 2.4 MB)
        PE :  mm(u,g) ps[bank] = V^T @ xs[:, g]      (vertical pass, 11 groups)
        ACT:  cp(u,g) ts[u%4][:, g] <- ps[bank]      (PSUM -> SBUF)
        DVE:  op1  xs[96:5472] = beta*xs + ts[0:5376]        (v = 2x + t@w-1)
              op2  ts[96:5472] = xs[96:5472] - ts[192:5568]  (out = v - t@w+1)
              drain -> inc dve sem
        SP :  O(u)  ts[96:5472] -> out rows/cols     (SBUF -> HBM)
    """
    from concourse import bass, mybir

    f32 = mybir.dt.float32
    nc = bass.Bass(debug=False)
    x_d = nc.declare_dram_parameter("x", [ROWS, SLAB], f32, isOutput=False)
    v_d = nc.declare_dram_parameter("vmat", [H, H], f32, isOutput=False)
    out_d = nc.declare_dram_parameter("out", [ROWS, FS], f32, isOutput=True)

    WHALF = W // 2            # 56 output columns per unit
    USLAB = (WHALF + 2) * CH  # 5568 slab floats (1 w-col halo each side)
    UINT = WHALF * CH         # 5376 interior floats
    NU = IMGS_PER_CORE * 2    # 8 units
    NS = 4                    # slab sets in flight

    groups = []
    off = 0
    while off < USLAB:
        n = min(MM_N, USLAB - off)
        groups.append((off, n))
        off += n
    n_g = len(groups)  # 11

    vt = nc.alloc_sbuf_tensor("vt", [H, H], f32)
    xs = [nc.alloc_sbuf_tensor(f"xs{k}", [H, USLAB], f32) for k in range(NS)]
    ts = [nc.alloc_sbuf_tensor(f"ts{k}", [H, USLAB], f32) for k in range(NS)]
    NB = 8
    ps = [nc.alloc_psum_tensor(f"ps{b}", [H, MM_N], f32) for b in range(NB)]

    def unit_rows(u):
        i = u // 2
        return i * H, (i + 1) * H

    def unit_slab_col(u):
        # start column of the unit's slab inside the padded x row [ROWS, SLAB]
        return (u % 2) * WHALF * CH  # 0 or 5376

    from contextlib import ExitStack

    with (
        nc.Block(no_gpsimd_drain=True) as block,
        nc.semaphore("s_vt") as s_vt,
        nc.semaphore("s_pe") as s_pe,
        nc.semaphore("s_act") as s_act,
        nc.semaphore("s_dve") as s_dve,
        ExitStack() as _sems,
    ):
        # Per-slab-set DMA completion semaphores.  A single cumulative DMA
        # semaphore would race: concurrent DMAs can complete out of issue
        # order, so "sem >= 16*(u+1)" could be satisfied by a LATER unit's
        # transfer while unit u's data is still in flight.  Per-set sems are
        # safe because successive users of one set never overlap in flight.
        s_din = [_sems.enter_context(nc.semaphore(f"s_din{k}")) for k in range(NS)]
        s_din2 = [_sems.enter_context(nc.semaphore(f"s_dinb{k}")) for k in range(NS)]
        s_dout = [_sems.enter_context(nc.semaphore(f"s_dout{k}")) for k in range(NS)]

        @block.sync
        def _(sp: bass.BassEngine):
            sp.dma_start(out=vt[:, :], in_=v_d[:, :]).then_inc(s_vt, 16)

            # loads are split in two halves on separate sems so the PE can
            # start on the first half; LSPLIT is a matmul-group boundary
            LSPLIT = 5 * MM_N  # 2560

            def load(u):
                r0, r1 = unit_rows(u)
                c0 = unit_slab_col(u)
                sp.dma_start(
                    out=xs[u % NS][:, 0:LSPLIT], in_=x_d[r0:r1, c0 : c0 + LSPLIT]
                ).then_inc(s_din[u % NS], 16)
                sp.dma_start(
                    out=xs[u % NS][:, LSPLIT:USLAB],
                    in_=x_d[r0:r1, c0 + LSPLIT : c0 + USLAB],
                ).then_inc(s_din2[u % NS], 16)

            for u in range(min(NS, NU)):
                load(u)
            for u in range(NU):
                r0, r1 = unit_rows(u)
                oc0 = (u % 2) * UINT
                # store unit u once its DVE drain fired
                sp.wait_ge(s_dve, u + 1)
                sp.dma_start(
                    out=out_d[r0:r1, oc0 : oc0 + UINT],
                    in_=ts[u % NS][:, PAD : PAD + UINT],
                ).then_inc(s_dout[u % NS], 16)
                nxt = u + NS
                if nxt < NU:
                    # reload xs[u%NS]: PE reads of unit u must be done (DVE
                    # covered by the store wait above)
                    sp.wait_ge(s_pe, n_g * (u + 1))
                    load(nxt)
            for k in range(NS):
                sp.wait_ge(s_dout[k], 16 * (NU // NS))

        @block.tensor
        def _(pe: bass.BassEngine):
            pe.wait_ge(s_vt, 16)
            for u in range(NU):
                pe.wait_ge(s_din[u % NS], 16 * (u // NS + 1))
                for g, (goff, gn) in enumerate(groups):
                    if g == 5:  # groups 5.. read past LSPLIT
                        pe.wait_ge(s_din2[u % NS], 16 * (u // NS + 1))
                    idx = u * n_g + g
                    if idx >= NB:
                        # psum bank reuse: the copy that read it must be done
                        pe.wait_ge(s_act, idx - NB + 1)
                    pe.matmul(
                        out=ps[idx % NB][0:H, 0:gn],
                        lhsT=vt[:, :],
                        rhs=xs[u % NS][:, goff : goff + gn],
                        start=True,
                        stop=True,
                    ).then_inc(s_pe, 1)

        @block.scalar
        def _(act: bass.BassEngine):
            for u in range(NU):
                if u >= NS:
                    # ts slab reuse: unit u-NS's DVE write and store DMA done
                    act.wait_ge(s_dve, u - NS + 1)
                    act.wait_ge(s_dout[u % NS], 16 * (u // NS))
                for g, (goff, gn) in enumerate(groups):
                    idx = u * n_g + g
                    act.wait_ge(s_pe, idx + 1)
                    act.copy(
                        out=ts[u % NS][:, goff : goff + gn],
                        in_=ps[idx % NB][0:H, 0:gn],
                    ).then_inc(s_act, 1)

        @block.vector
        def _(dve: bass.BassEngine):
            for u in range(NU):
                # all matmul groups of unit u must have read xs before op1
                # overwrites it, and all copies must have produced ts
                dve.wait_ge(s_pe, n_g * (u + 1))
                dve.wait_ge(s_act, n_g * (u + 1))
                dve.scalar_tensor_tensor(
                    out=xs[u % NS][:, PAD : PAD + UINT],
                    in0=xs[u % NS][:, PAD : PAD + UINT],
                    scalar=float(beta),
                    in1=ts[u % NS][:, 0:UINT],
                    op0=mybir.AluOpType.mult,
                    op1=mybir.AluOpType.add,
                )
                dve.tensor_tensor(
                    out=ts[u % NS][:, PAD : PAD + UINT],
                    in0=xs[u % NS][:, PAD : PAD + UINT],
                    in1=ts[u % NS][:, 2 * PAD : 2 * PAD + UINT],
                    op=mybir.AluOpType.subtract,
                )
                dve.drain().then_inc(s_dve, 1)

    return nc


def _stencil_params(kern):
    """Validate the depthwise kernel and extract (vertical profile a, beta).

    Requires: channels identical, k[:,2] == -k[:,0], k[0,1] == k[2,1] == 0.
    Returns (a, beta) with a = k[:,0] (vertical mixing profile) and
    beta = k[1,1] + 1 (center coefficient incl. the residual).
    """
    k = np.asarray(kern, dtype=np.float32)
    if k.ndim != 4 or k.shape != (3, 3, 1, CH):
        return None
    if not np.all(k == k[:, :, :, :1]):
        return None
    k2 = k[:, :, 0, 0]
    if not (np.all(k2[:, 2] == -k2[:, 0]) and k2[0, 1] == 0 and k2[2, 1] == 0):
        return None
    return k2[:, 0].copy(), float(k2[1, 1]) + 1.0


def _numpy_fallback(x, kern):
    """Straightforward shifted-add implementation (safety net only)."""
    k = np.asarray(kern, dtype=np.float32)[:, :, 0, :]  # (3,3,CH)
    xp = np.pad(x, ((0, 0), (1, 1), (1, 1), (0, 0)))
    out = x.astype(np.float32).copy()
    for dh in range(3):
        for dw in range(3):
            out += k[dh, dw] * xp[:, dh : dh + H, dw : dw + W, :]
    return out


def _ensure_ntff_hook():
    """The agent image's antenv lacks axon_hooks; synthesize it so
    run_bass_kernel_spmd(trace=True) can reach the NTFF profiler."""
    import types

    if "antenv.axon_hooks" in sys.modules:
        return
    import antenv

    mod = types.ModuleType("antenv.axon_hooks")
    state = {}
    mod.set_axon_ntff_profile_hook = lambda h: state.__setitem__("h", h)
    mod.get_axon_ntff_profile_hook = lambda: state.get("h")
    sys.modules["antenv.axon_hooks"] = mod
    antenv.axon_hooks = mod
    try:
        if "/root/.axon_site" not in sys.path:
            sys.path.insert(0, "/root/.axon_site")
        from trn_agent_boot.trn_boot import _ntff_profile_via_ctypes

        hook = _ntff_profile_via_ctypes("/opt/axon/libaxon_pjrt.so")
        if hook is not None:
            mod.set_axon_ntff_profile_hook(hook)
    except Exception:
        pass


def _run_on_hw(x, a, beta, trace=False):
    global LAST_RESULTS
    if trace:
        _ensure_ntff_hook()
    from concourse.bass_utils import run_bass_kernel_spmd

    # vertical banded matrix: V[i, j] = coeff of x-row i in t-row j
    V = np.zeros((H, H), dtype=np.float32)
    idx = np.arange(H)
    V[idx[:-1] + 1, idx[:-1]] += a[2]   # i = j+1
    V[idx, idx] += a[1]                 # i = j
    V[idx[1:] - 1, idx[1:]] += a[0]     # i = j-1

    key = (a.tobytes(), float(beta))
    if key not in _CACHE:
        _CACHE[key] = _build_bass(beta)
    nc = _CACHE[key]

    # host-side zero padding of one w column on each side (pads the slab so
    # the device needs no memsets)
    xp = np.zeros((N_CORES, ROWS, SLAB), dtype=np.float32)
    xp[:, :, PAD : PAD + FS] = x.reshape(N_CORES, ROWS, FS)
    in_maps = [{"x": xp[c], "vmat": V} for c in range(N_CORES)]
    res = run_bass_kernel_spmd(nc, in_maps, list(range(N_CORES)), trace=trace)
    LAST_RESULTS = res
    out = np.stack([res.results[c]["out"] for c in range(N_CORES)])
    return out.reshape(N, H, W, CH)


def kernel(x, kernel=None, _trace=False, **_unused):
    x = np.ascontiguousarray(np.asarray(x, dtype=np.float32))
    assert x.shape == (N, H, W, CH), f"unexpected x shape {x.shape}"
    if kernel is None:
        base = np.array(
            [[1.0, 0.0, -1.0], [0.0, 1.0, 0.0], [-1.0, 0.0, 1.0]], dtype=np.float32
        )
        kernel = np.tile(base[:, :, None, None], (1, 1, 1, CH))
    params = _stencil_params(kernel)
    if params is None:
        return _numpy_fallback(x, kernel)
    a, beta = params
    return _run_on_hw(x, a, beta, trace=_trace)


if __name__ == "__main__":
    xs = np.random.randn(N, H, W, CH).astype(np.float32)
    out = kernel(xs)
    print(out.shape, out.dtype)
